# revision 1
# baseline (speedup 1.0000x reference)
"""Trainium2 Bass kernel for nn_BoundaryDistillationLoss.

loss = mean((|grad(softmax(s))| - |grad(softmax(t))|)^2) with depthwise 3x3
Sobel gradients. Expanded as  [ sum(qs) + sum(qt) - 2*sum(sqrt(qs*qt)) ] / N
where q = gx^2 + gy^2, so no per-tensor sqrt is needed (one sqrt per pair).

Data parallel over B*H rows (2048) across 8 cores; each core gets a
(C, 258, W) halo-padded shard per tensor.  On-chip layout: h-rows on SBUF
partitions, (c, w) on the free dim.  The Sobel y-taps are banded 128x128
matmuls on the tensor engine; the x-taps are folded into the same matmuls
via +-1-shifted rhs views of a W-padded prob slab (so conv zero-padding is
exact with no edge fixups).  The 4-row tail that doesn't fit the 126-row
slab tiling is processed in a packed layout: partitions = (channel, row),
student/teacher side by side in the free dim, so it costs ~1/20 of a slab
instead of a full one.

Custom DVE ops (SQSUM/SQADD) fuse q = a^2 + b^2 with a running free-dim
sum; squares are split between ScalarE (Square activation reading PSUM)
and VectorE to balance engine load.
"""

import numpy as np
from contextlib import ExitStack

import concourse.bass as bass
import concourse.bacc as bacc
import concourse.mybir as mybir
import concourse.tile as tile
from concourse import bass_utils
import concourse.dve_ops as dve_ops
from concourse.dve_spec import C0 as _C0, Spec as _Spec, Src0 as _Src0, \
    Src1 as _Src1, lower as _dve_lower, sq as _dve_sq
from concourse.dve_uop import DveOpSpec as _DveOpSpec
from operator import add as _op_add


def _register_custom(name, body, reference):
    for o in dve_ops.OPS:
        if o.name == name:
            return o
    spec = _Spec(body=body, accum=_op_add, accum_init=_C0, reference=reference)
    row = 1 + len(dve_ops.OPS)
    assert row < 0x20
    dve_ops._SUB_OPCODE_FOR_NAME[name] = row
    shas = {}
    for ver in ("v3", "v4"):
        try:
            uops = _dve_lower(spec, ver=ver)
            shas[ver] = _DveOpSpec(name=name, opcode=row, uops=uops,
                                   rd1_en=True).sha(ver)
        except Exception:
            pass
    op = dve_ops.DveOp(name, spec, subdim=False, uops_sha=shas)
    dve_ops.OPS.append(op)
    dve_ops.CUSTOM_DVE_SPECS[name] = spec
    return op


def _ref_sqsum(in0, in1, c0, c1, c2):
    b = (in0.astype(np.float32) ** 2 + in1.astype(np.float32) ** 2).astype(np.float32)
    return b, c0 + b.reshape(b.shape[0], -1).sum(axis=-1, keepdims=True)


def _ref_sqadd(in0, in1, c0, c1, c2):
    b = (in0.astype(np.float32) ** 2 + in1.astype(np.float32)).astype(np.float32)
    return b, c0 + b.reshape(b.shape[0], -1).sum(axis=-1, keepdims=True)


SQSUM = _register_custom("SQSUM_ANT", _dve_sq(_Src0) + _dve_sq(_Src1), _ref_sqsum)
SQADD = _register_custom("SQADD_ANT", _dve_sq(_Src0) + _Src1, _ref_sqadd)

F32 = mybir.dt.float32
BF16 = mybir.dt.bfloat16
NP_BF16 = mybir.dt.np(BF16)

# Problem constants (hardcoded per spec: nn_BoundaryDistillationLoss_87230785781774)
B, C, H, W = 4, 19, 512, 1024
NCORES = 8
ROWS_PER_CORE = (B * H) // NCORES          # 256
HIN = ROWS_PER_CORE + 2                    # 258 (one halo row each side)
# main slabs: (in_row_start, n_in_rows, n_out_rows); out = in - 2 (valid conv)
MAIN_SLABS = ((0, 128, 126), (126, 128, 126))
REM = (252, 6)                             # packed tail: in rows 252..257 -> out 252..255
EXP_CHUNK = 4                              # channels per DMA+exp instruction


def _shifted_band(a, n, nfull=128):
    """lhsT [nfull, nfull] with lhsT[k, m] = a[m+1, k] (out row m = conv row
    m+1 so DVE consumers start at partition 0); a is [n, n]."""
    t = np.zeros((nfull, nfull), np.float32)
    t[:n, : n - 1] = a.T[:, 1:]
    return t


def _base_bands(n):
    A_s = np.zeros((n, n), np.float32)
    A_d = np.zeros((n, n), np.float32)
    i = np.arange(n)
    A_s[i, i] = 2.0
    A_s[i[:-1], i[:-1] + 1] = 1.0
    A_s[i[1:], i[1:] - 1] = 1.0
    A_d[i[:-1], i[:-1] + 1] = 1.0
    A_d[i[1:], i[1:] - 1] = -1.0
    return A_s, A_d


def _band_weights(c_dim=C, blk=6):
    A_s, A_d = _base_bands(128)
    out = {
        "w_sp": _shifted_band(A_s, 128),
        "w_sn": _shifted_band(-A_s, 128),
        "w_d": _shifted_band(A_d, 128),
        "w_d2": _shifted_band(2.0 * A_d, 128),
        "ident": np.eye(128, dtype=np.float32),
    }
    # packed-remainder block-diagonal bands: c_dim blocks of blk rows
    a_s, a_d = _base_bands(blk)
    npk = c_dim * blk
    assert npk <= 128
    for name, a in (("w_rsp", a_s), ("w_rsn", -a_s), ("w_rd", a_d),
                    ("w_rd2", 2.0 * a_d)):
        m = np.zeros((128, 128), np.float32)
        sb = _shifted_band(a, blk, blk)
        # out rows blk-2.. would be partial convs of the halo row; the
        # consumers read all packed partitions, so force them to zero
        sb[:, blk - 2 :] = 0.0
        for cblk in range(c_dim):
            m[cblk * blk : (cblk + 1) * blk, cblk * blk : (cblk + 1) * blk] = sb
        out[name] = m
    w_sel = np.zeros((128, 128), np.float32)   # z[i] = sum_c exp[c*blk+i]
    w_rep = np.zeros((128, 128), np.float32)   # rep[c*blk+i] = r[i]
    for cblk in range(c_dim):
        for i in range(blk):
            w_sel[cblk * blk + i, i] = 1.0
            w_rep[i, cblk * blk + i] = 1.0
    out["w_sel"] = w_sel
    out["w_rep"] = w_rep
    return {k: v.astype(NP_BF16) for k, v in out.items()}


def acc_layout(main_slabs, c, nwh=2, rem=True):
    nq = len(main_slabs) * 2 * c * nwh + (2 * nwh if rem else 0)
    ns = len(main_slabs) * ((c + 1) // 2) + (1 if rem else 0)
    return nq, ns


def build_nc(c_dim=C, w_dim=W, hin=HIN, main_slabs=MAIN_SLABS, rem=REM):
    nwh = max(1, w_dim // 512)
    wc = w_dim // nwh
    nq, ns = acc_layout(main_slabs, c_dim, nwh, rem is not None)
    nacc = nq + ns
    blk = rem[1] if rem is not None else 6
    npk = c_dim * blk

    nc = bacc.Bacc("TRN2", target_bir_lowering=False)
    xs = nc.dram_tensor("xs", [c_dim, hin, w_dim], F32, kind="ExternalInput")
    xt = nc.dram_tensor("xt", [c_dim, hin, w_dim], F32, kind="ExternalInput")
    wnames = ("w_sp", "w_sn", "w_d", "w_d2", "ident",
              "w_rsp", "w_rsn", "w_rd", "w_rd2", "w_sel", "w_rep")
    wts = {n: nc.dram_tensor(n, [128, 128], BF16, kind="ExternalInput")
           for n in wnames}
    acc_out = nc.dram_tensor("acc", [128, nacc], F32, kind="ExternalOutput")

    x_dram = (xs, xt)
    EXP = mybir.ActivationFunctionType.Exp
    SQRT = mybir.ActivationFunctionType.Sqrt
    SQUARE = mybir.ActivationFunctionType.Square

    qcol = iter(range(nq))
    scol = iter(range(nq, nacc))

    with ExitStack() as ctx:
        tc = ctx.enter_context(tile.TileContext(nc))
        sb = ctx.enter_context(tc.tile_pool(name="sb", bufs=2))
        consts = ctx.enter_context(tc.tile_pool(name="consts", bufs=1))
        psum = ctx.enter_context(tc.tile_pool(name="psum", bufs=1, space="PSUM"))

        w_sb = {}
        for name in wnames:
            t = consts.tile([128, 128], BF16, tag=name)
            nc.sync.dma_start(out=t, in_=wts[name][:, :])
            w_sb[name] = t
        acc_sb = consts.tile([128, nacc], F32, tag="acc")
        nc.vector.memset(acc_sb[:, :], 0.0)

        # ~4us of dummy matmuls right after the weight DMAs: trips the PE
        # HAM un-throttle (4096-cycle activity window) before real work
        # arrives, so the first conv matmuls run at 2.4 GHz instead of 1.2
        warm = psum.tile([128, 512], F32, tag="z", bufs=2)
        for wi in range(24):
            nc.tensor.matmul(warm[:, 0:128], lhsT=w_sb["ident"][:, :],
                             rhs=w_sb["ident"][:, :], start=True, stop=True)

        chunks = []
        c0 = 0
        while c0 < c_dim:
            cn = min(EXP_CHUNK, c_dim - c0)
            chunks.append((c0, cn))
            c0 += cn

        def squares(nout, cc, wh, gx, gy, q, g2, h2, b0):
            """q[0:nout, b0:b0+wc] = gx^2 + gy^2 (+ sum into a fresh acc col)."""
            col = next(qcol)
            acc_col = acc_sb[0:nout, col : col + 1]
            if (cc + wh) % 3 == 0:
                nc.vector.tensor_copy(out=h2[0:nout, b0 : b0 + wc],
                                      in_=gy[0:nout, :])
                nc.vector._custom_dve(
                    SQSUM, out=q[0:nout, b0 : b0 + wc], in0=gx[0:nout, :],
                    in1=h2[0:nout, b0 : b0 + wc], s0=0.0, accum_out=acc_col)
            else:
                nc.scalar.activation(out=g2[0:nout, b0 : b0 + wc],
                                     in_=gx[0:nout, :], func=SQUARE)
                nc.vector._custom_dve(
                    SQADD, out=q[0:nout, b0 : b0 + wc], in0=gy[0:nout, :],
                    in1=g2[0:nout, b0 : b0 + wc], s0=0.0, accum_out=acc_col)

        def conv_mms(wn_sp, wn_sn, wn_d, wn_d2, nin, ps_view, b0):
            """gx/gy psum tiles for one (c, T, wh); ps_view = [128, w+4] bf16."""
            gx = psum.tile([128, wc], F32, tag="gx", bufs=3)
            nc.tensor.matmul(gx[:, :], lhsT=w_sb[wn_sp][0:nin, :],
                             rhs=ps_view[0:nin, b0 + 3 : b0 + 3 + wc],
                             start=True, stop=False)
            nc.tensor.matmul(gx[:, :], lhsT=w_sb[wn_sn][0:nin, :],
                             rhs=ps_view[0:nin, b0 + 1 : b0 + 1 + wc],
                             start=False, stop=True)
            gy = psum.tile([128, wc], F32, tag="gy", bufs=3)
            nc.tensor.matmul(gy[:, :], lhsT=w_sb[wn_d][0:nin, :],
                             rhs=ps_view[0:nin, b0 + 1 : b0 + 1 + wc],
                             start=True, stop=False)
            nc.tensor.matmul(gy[:, :], lhsT=w_sb[wn_d2][0:nin, :],
                             rhs=ps_view[0:nin, b0 + 2 : b0 + 2 + wc],
                             start=False, stop=False)
            nc.tensor.matmul(gy[:, :], lhsT=w_sb[wn_d][0:nin, :],
                             rhs=ps_view[0:nin, b0 + 3 : b0 + 3 + wc],
                             start=False, stop=True)
            return gx, gy

        # ---------------- main slabs ----------------
        for si, (r0, nin, nout) in enumerate(main_slabs):
            pslabs = []
            for ti in range(2):
                ps = sb.tile([128, c_dim, w_dim + 4], BF16, tag=f"pslab{ti}",
                             bufs=1)
                pslabs.append(ps)
                nc.vector.memset(
                    ps[0:nin, :, 1 : w_dim + 3 : w_dim + 1], 0.0)
                for (cc0, cn) in chunks:
                    stg = sb.tile([128, cn, w_dim], F32, tag="stage", bufs=3)
                    nc.sync.dma_start(
                        out=stg[0:nin, :, :],
                        in_=x_dram[ti][cc0 : cc0 + cn, r0 : r0 + nin, :]
                        .rearrange("c h w -> h c w"))
                    nc.scalar.activation(
                        out=ps[0:nin, cc0 : cc0 + cn, 2 : 2 + w_dim],
                        in_=stg[0:nin, :, :], func=EXP)
                r32 = sb.tile([128, w_dim], F32, tag="r32", bufs=2)
                for wh in range(nwh):
                    z = psum.tile([128, wc], F32, tag="z", bufs=2)
                    for cc in range(c_dim):
                        nc.tensor.matmul(
                            z[0:nin, :], lhsT=w_sb["ident"][0:nin, 0:nin],
                            rhs=ps[0:nin, cc, 2 + wh * wc : 2 + (wh + 1) * wc],
                            start=(cc == 0), stop=(cc == c_dim - 1))
                    nc.vector.reciprocal_approx_fast(
                        out=r32[0:nin, wh * wc : (wh + 1) * wc], in_=z[0:nin, :])
                r16 = sb.tile([128, w_dim], BF16, tag="r16", bufs=2)
                nc.vector.tensor_copy(out=r16[0:nin, :], in_=r32[0:nin, :])
                for cc in range(c_dim):
                    nc.vector.tensor_mul(
                        out=ps[0:nin, cc, 2 : 2 + w_dim],
                        in0=ps[0:nin, cc, 2 : 2 + w_dim],
                        in1=r16[0:nin, 0:w_dim])

            p16 = None
            for cc in range(c_dim):
                q_tiles = []
                for ti in range(2):
                    g2 = sb.tile([128, w_dim], BF16, tag=f"g2_{ti}", bufs=2)
                    h2 = sb.tile([128, w_dim], BF16, tag=f"h2_{ti}", bufs=2)
                    q = sb.tile([128, w_dim], BF16, tag=f"q_{ti}", bufs=2)
                    for wh in range(nwh):
                        gx, gy = conv_mms("w_sp", "w_sn", "w_d", "w_d2", nin,
                                          pslabs[ti][:, cc, :], wh * wc)
                        squares(nout, cc, wh, gx, gy, q, g2, h2, wh * wc)
                    q_tiles.append(q)
                if cc % 2 == 0:
                    p16 = sb.tile([128, 2, w_dim], BF16, tag="p16", bufs=2)
                nc.gpsimd.tensor_mul(out=p16[0:nout, cc % 2, :],
                                     in0=q_tiles[0][0:nout, :],
                                     in1=q_tiles[1][0:nout, :])
                if cc % 2 == 1 or cc == c_dim - 1:
                    nsl = cc % 2 + 1
                    psq = sb.tile([128, 2, w_dim], BF16, tag="psq", bufs=2)
                    col = next(scol)
                    nc.scalar.activation(
                        out=psq[0:nout, 0:nsl, :],
                        in_=p16[0:nout, 0:nsl, :], func=SQRT,
                        accum_out=acc_sb[0:nout, col : col + 1])

        # ---------------- packed remainder ----------------
        if rem is not None:
            r0 = rem[0]
            rps = sb.tile([128, 2, w_dim + 4], BF16, tag="rem_ps", bufs=1)
            nc.vector.memset(rps[0:npk, :, 1 : w_dim + 3 : w_dim + 1], 0.0)
            stg = sb.tile([128, 2, w_dim], F32, tag="rem_stage", bufs=1)
            for ti in range(2):
                for cc in range(c_dim):
                    nc.sync.dma_start(
                        out=stg[cc * blk : (cc + 1) * blk, ti, :],
                        in_=x_dram[ti][cc, r0 : r0 + blk, :])
            nc.scalar.activation(out=rps[0:npk, :, 2 : 2 + w_dim],
                                 in_=stg[0:npk, :, :], func=EXP)
            for ti in range(2):
                r32 = sb.tile([128, w_dim], F32, tag="r32", bufs=2)
                for wh in range(nwh):
                    z = psum.tile([128, wc], F32, tag="z", bufs=2)
                    nc.tensor.matmul(
                        z[0:blk, :], lhsT=w_sb["w_sel"][0:npk, 0:blk],
                        rhs=rps[0:npk, ti, 2 + wh * wc : 2 + (wh + 1) * wc],
                        start=True, stop=True)
                    nc.vector.reciprocal_approx_fast(
                        out=r32[0:blk, wh * wc : (wh + 1) * wc], in_=z[0:blk, :])
                r16 = sb.tile([128, w_dim], BF16, tag="r16", bufs=2)
                nc.vector.tensor_copy(out=r16[0:blk, :], in_=r32[0:blk, :])
                for wh in range(nwh):
                    rrep = psum.tile([128, wc], F32, tag="gx", bufs=3)
                    nc.tensor.matmul(
                        rrep[0:npk, :], lhsT=w_sb["w_rep"][0:blk, 0:npk],
                        rhs=r16[0:blk, wh * wc : (wh + 1) * wc],
                        start=True, stop=True)
                    nc.vector.tensor_mul(
                        out=rps[0:npk, ti, 2 + wh * wc : 2 + (wh + 1) * wc],
                        in0=rps[0:npk, ti, 2 + wh * wc : 2 + (wh + 1) * wc],
                        in1=rrep[0:npk, :])
            q_tiles = []
            for ti in range(2):
                g2 = sb.tile([128, w_dim], BF16, tag="g2_0", bufs=2)
                h2 = sb.tile([128, w_dim], BF16, tag="h2_0", bufs=2)
                q = sb.tile([128, w_dim], BF16, tag=f"q_{ti}", bufs=2)
                for wh in range(nwh):
                    gx, gy = conv_mms("w_rsp", "w_rsn", "w_rd", "w_rd2", npk,
                                      rps[:, ti, :], wh * wc)
                    squares(npk, 1 + wh, wh, gx, gy, q, g2, h2, wh * wc)
                q_tiles.append(q)
            p16 = sb.tile([128, 2, w_dim], BF16, tag="p16", bufs=2)
            nc.gpsimd.tensor_mul(out=p16[0:npk, 0, :], in0=q_tiles[0][0:npk, :],
                                 in1=q_tiles[1][0:npk, :])
            psq = sb.tile([128, 2, w_dim], BF16, tag="psq", bufs=2)
            col = next(scol)
            nc.scalar.activation(out=psq[0:npk, 0, :], in_=p16[0:npk, 0, :],
                                 func=SQRT,
                                 accum_out=acc_sb[0:npk, col : col + 1])

        nc.sync.dma_start(out=acc_out[:, :], in_=acc_sb[:, :])
    if not nc.is_finalized():
        nc.finalize()
    return nc


def shard_inputs(student_logits, teacher_logits, c_dim=C, h_dim=H, w_dim=W,
                 ncores=NCORES):
    """Full (B,C,H,W) fp32 -> per-core dicts with (C, rows+2, W) halo shards."""
    b_dim = student_logits.shape[0]
    rows = (b_dim * h_dim) // ncores
    in_maps = []
    wts = _band_weights()
    for k in range(ncores):
        g0 = k * rows
        bi, h0 = g0 // h_dim, g0 % h_dim
        m = {}
        for name, x in (("xs", student_logits), ("xt", teacher_logits)):
            img = x[bi]                                    # (C, H, W)
            sh = np.zeros((c_dim, rows + 2, w_dim), np.float32)
            lo, hi = h0 - 1, h0 + rows + 1
            src_lo, src_hi = max(lo, 0), min(hi, h_dim)
            sh[:, src_lo - lo : src_lo - lo + (src_hi - src_lo), :] = \
                np.asarray(img[:, src_lo:src_hi, :], np.float32)
            m[name] = sh
        for name, wv in wts.items():
            m[name] = wv
        in_maps.append(m)
    return in_maps


_NC_CACHE = {}


def _get_nc():
    key = "full"
    if key not in _NC_CACHE:
        _NC_CACHE[key] = build_nc()
    return _NC_CACHE[key]


def run_on_cores(in_maps, trace=False, **kw):
    nc = _get_nc()
    res = bass_utils.run_bass_kernel_spmd(
        nc, in_maps, core_ids=list(range(len(in_maps))), trace=trace, **kw
    )
    return res


def finish(results, main_slabs=MAIN_SLABS, c_dim=C, n_total=None, nwh=2,
           rem=True):
    nq, ns = acc_layout(main_slabs, c_dim, nwh, rem)
    if n_total is None:
        n_total = B * C * H * W
    tq = 0.0
    tcross = 0.0
    for r in results:
        a = np.asarray(r["acc"], np.float64)
        tq += a[:, :nq].sum()
        tcross += a[:, nq:].sum()
    return np.float32((tq - 2.0 * tcross) / n_total)


def kernel(student_logits, teacher_logits):
    in_maps = shard_inputs(np.asarray(student_logits), np.asarray(teacher_logits))
    res = run_on_cores(in_maps)
    return finish(res.results)



# revision 4
# speedup vs baseline: 1.0207x; 1.0207x over previous
"""Trainium2 Bass kernel for nn_BoundaryDistillationLoss.

loss = mean((|grad(softmax(s))| - |grad(softmax(t))|)^2) with depthwise 3x3
Sobel gradients, expanded as [ sum(qs) + sum(qt) - 2*sum(sqrt(qs*qt)) ] / N
with q = gx^2 + gy^2.

v6 layout: host pre-casts inputs to bf16 and rearranges each core's shard to
(h, c, w) so every DMA is contiguous.  2048 rows data-parallel over 8 cores;
per core two 128-row slabs (126 output rows each) plus a packed 6-row tail.
On-chip: h-rows on partitions, (c, w) on the free dim, per-4-channel chunk
tiles in a ring so slab N+1's DMA/exp/z-sum/normalize overlaps slab N's conv.
Sobel row-taps are banded 128x128 matmuls; col-taps are +-1-shifted rhs views
of a zero-padded slab.  Per (channel, w-half) all four conv outputs go to one
4-bank PSUM group [gxs|gxt|gys|gyt]: ScalarE squares the gx half while
VectorE runs a fused square-add (SQADD) on the gy half in different banks,
GPSIMD multiplies qs*qt, and one big in-place SQRT per slab (with free
accumulate) produces the cross term without exp<->sqrt table thrashing.
"""

import numpy as np
from contextlib import ExitStack

import concourse.bass as bass
import concourse.bacc as bacc
import concourse.mybir as mybir
import concourse.tile as tile
from concourse import bass_utils
import concourse.dve_ops as dve_ops
from concourse.dve_spec import C0 as _C0, Spec as _Spec, Src0 as _Src0, \
    Src1 as _Src1, lower as _dve_lower, sq as _dve_sq
from concourse.dve_uop import DveOpSpec as _DveOpSpec
from operator import add as _op_add


def _register_custom(name, body, reference):
    for o in dve_ops.OPS:
        if o.name == name:
            return o
    spec = _Spec(body=body, accum=_op_add, accum_init=_C0, reference=reference)
    row = 1 + len(dve_ops.OPS)
    assert row < 0x20
    dve_ops._SUB_OPCODE_FOR_NAME[name] = row
    shas = {}
    for ver in ("v3", "v4"):
        try:
            uops = _dve_lower(spec, ver=ver)
            shas[ver] = _DveOpSpec(name=name, opcode=row, uops=uops,
                                   rd1_en=True).sha(ver)
        except Exception:
            pass
    op = dve_ops.DveOp(name, spec, subdim=False, uops_sha=shas)
    dve_ops.OPS.append(op)
    dve_ops.CUSTOM_DVE_SPECS[name] = spec
    return op


def _ref_sqsum(in0, in1, c0, c1, c2):
    b = (in0.astype(np.float32) ** 2 + in1.astype(np.float32) ** 2).astype(np.float32)
    return b, c0 + b.reshape(b.shape[0], -1).sum(axis=-1, keepdims=True)


def _ref_sqadd(in0, in1, c0, c1, c2):
    b = (in0.astype(np.float32) ** 2 + in1.astype(np.float32)).astype(np.float32)
    return b, c0 + b.reshape(b.shape[0], -1).sum(axis=-1, keepdims=True)


SQSUM = _register_custom("SQSUM_ANT", _dve_sq(_Src0) + _dve_sq(_Src1), _ref_sqsum)
SQADD = _register_custom("SQADD_ANT", _dve_sq(_Src0) + _Src1, _ref_sqadd)

F32 = mybir.dt.float32
BF16 = mybir.dt.bfloat16
NP_BF16 = mybir.dt.np(BF16)

# Problem constants (hardcoded: nn_BoundaryDistillationLoss_87230785781774)
B, C, H, W = 4, 19, 512, 1024
NCORES = 8
ROWS_PER_CORE = (B * H) // NCORES          # 256
HIN = ROWS_PER_CORE + 2                    # 258 (one halo row each side)
SLABS = ((0, 128, 126), (126, 128, 126))   # (in_row_start, n_in, n_out)
REM = (252, 6)                             # packed tail rows 252..257 -> 252..255
CHUNKS = ((0, 4), (4, 4), (8, 4), (12, 4), (16, 3))
WC = 512
PCH_BUFS = 7                               # per-ti chunk-tile ring depth

NQ = 2 * C * 2 + 4                         # SQADD accum cols: (slab,cc,wh) + rem
NS = 2 + 1                                 # per-slab sqrt cols + rem sqrt col
NACC = NQ + NS


def _shifted_band(a, n, nfull=128):
    """lhsT [nfull, nfull] with lhsT[k, m] = a[m+1, k] (out row m = conv row
    m+1 so consumers start at partition 0); a is [n, n]."""
    t = np.zeros((nfull, nfull), np.float32)
    t[:n, : n - 1] = a.T[:, 1:]
    return t


def _base_bands(n):
    A_s = np.zeros((n, n), np.float32)
    A_d = np.zeros((n, n), np.float32)
    i = np.arange(n)
    A_s[i, i] = 2.0
    A_s[i[:-1], i[:-1] + 1] = 1.0
    A_s[i[1:], i[1:] - 1] = 1.0
    A_d[i[:-1], i[:-1] + 1] = 1.0
    A_d[i[1:], i[1:] - 1] = -1.0
    return A_s, A_d


def _band_weights(c_dim=C, blk=6):
    A_s, A_d = _base_bands(128)
    out = {
        "w_sp": _shifted_band(A_s, 128),
        "w_sn": _shifted_band(-A_s, 128),
        "w_d": _shifted_band(A_d, 128),
        "w_d2": _shifted_band(2.0 * A_d, 128),
        "ident": np.eye(128, dtype=np.float32),
    }
    a_s, a_d = _base_bands(blk)
    npk = c_dim * blk
    assert npk <= 128
    for name, a in (("w_rsp", a_s), ("w_rsn", -a_s), ("w_rd", a_d),
                    ("w_rd2", 2.0 * a_d)):
        m = np.zeros((128, 128), np.float32)
        sb_ = _shifted_band(a, blk, blk)
        sb_[:, blk - 2:] = 0.0
        for cblk in range(c_dim):
            m[cblk * blk: (cblk + 1) * blk, cblk * blk: (cblk + 1) * blk] = sb_
        out[name] = m
    w_sel = np.zeros((128, 128), np.float32)
    w_rep = np.zeros((128, 128), np.float32)
    for cblk in range(c_dim):
        for i in range(blk):
            w_sel[cblk * blk + i, i] = 1.0
            w_rep[i, cblk * blk + i] = 1.0
    out["w_sel"] = w_sel
    out["w_rep"] = w_rep
    return {k: v.astype(NP_BF16) for k, v in out.items()}


WNAMES = ("w_sp", "w_sn", "w_d", "w_d2", "ident",
          "w_rsp", "w_rsn", "w_rd", "w_rd2", "w_sel", "w_rep")


def build_nc():
    blk = REM[1]
    npk = C * blk

    nc = bacc.Bacc("TRN2", target_bir_lowering=False)
    xs = nc.dram_tensor("xs", [HIN, C, W], BF16, kind="ExternalInput")
    xt = nc.dram_tensor("xt", [HIN, C, W], BF16, kind="ExternalInput")
    wts = {n: nc.dram_tensor(n, [128, 128], BF16, kind="ExternalInput")
           for n in WNAMES}
    acc_out = nc.dram_tensor("acc", [128, NACC], F32, kind="ExternalOutput")

    x_dram = (xs, xt)
    EXP = mybir.ActivationFunctionType.Exp
    SQRT = mybir.ActivationFunctionType.Sqrt
    SQUARE = mybir.ActivationFunctionType.Square

    qcol = iter(range(NQ))
    scol = iter(range(NQ, NACC))

    with ExitStack() as ctx:
        tc = ctx.enter_context(tile.TileContext(nc))
        sb = ctx.enter_context(tc.tile_pool(name="sb", bufs=2))
        consts = ctx.enter_context(tc.tile_pool(name="consts", bufs=1))
        psum = ctx.enter_context(tc.tile_pool(name="psum", bufs=1, space="PSUM"))

        w_sb = {}
        for name in WNAMES:
            t = consts.tile([128, 128], BF16, tag=name)
            nc.sync.dma_start(out=t, in_=wts[name][:, :])
            w_sb[name] = t
        acc_sb = consts.tile([128, NACC], F32, tag="acc")
        nc.vector.memset(acc_sb[:, :], 0.0)

        # PE HAM warm-up: ~4us of dummy matmuls so real convs start at 2.4GHz
        warm = psum.tile([128, 2048], F32, tag="qg", bufs=2)
        for _ in range(24):
            nc.tensor.matmul(warm[:, 0:128], lhsT=w_sb["ident"][:, :],
                             rhs=w_sb["ident"][:, :], start=True, stop=True)

        MM = nc.tensor.matmul

        # ---------- build-phase helpers (per slab, tensor) ----------
        def emit_dma_exp(pmap, s, ti, ci):
            r0, nin, _ = SLABS[s]
            cc0, cn = CHUNKS[ci]
            t = sb.tile([128, 4, W + 4], BF16, tag=f"pch{ti}", bufs=PCH_BUFS)
            # zero the conv border cols (1 and W+2); cols 0 / W+3 unused
            nc.vector.memset(t[0:nin, 0:cn, 1: W + 3: W + 1], 0.0)
            nc.sync.dma_start(
                out=t[0:nin, 0:cn, 2: 2 + W],
                in_=x_dram[ti][r0: r0 + nin, cc0: cc0 + cn, :])
            nc.scalar.activation(out=t[0:nin, 0:cn, 2: 2 + W],
                                 in_=t[0:nin, 0:cn, 2: 2 + W], func=EXP)
            pmap[(ti, ci)] = t

        def emit_z(pmap, s, ti):
            _, nin, _ = SLABS[s]
            zt = psum.tile([128, 2048], F32, tag="qg", bufs=2)
            for wh in (0, 1):
                first = True
                for ci, (cc0, cn) in enumerate(CHUNKS):
                    t = pmap[(ti, ci)]
                    for c in range(cn):
                        MM(zt[0:nin, wh * WC: (wh + 1) * WC],
                           lhsT=w_sb["ident"][0:nin, 0:nin],
                           rhs=t[0:nin, c, 2 + wh * WC: 2 + (wh + 1) * WC],
                           start=first, stop=(ci == 4 and c == cn - 1))
                        first = False
            return zt

        def emit_recip(s, ti, zt):
            _, nin, _ = SLABS[s]
            r32 = sb.tile([128, W], F32, tag="r32", bufs=2)
            nc.vector.reciprocal_approx_fast(out=r32[0:nin, :],
                                             in_=zt[0:nin, 0:1024])
            r16 = sb.tile([128, W], BF16, tag="r16", bufs=2)
            nc.vector.tensor_copy(out=r16[0:nin, :], in_=r32[0:nin, :])
            return r16

        def emit_norm(pmap, s, ti, r16, cis):
            _, nin, _ = SLABS[s]
            for ci in cis:
                cc0, cn = CHUNKS[ci]
                t = pmap[(ti, ci)]
                for c in range(cn):
                    nc.vector.tensor_mul(out=t[0:nin, c, 2: 2 + W],
                                         in0=t[0:nin, c, 2: 2 + W],
                                         in1=r16[0:nin, :])

        # ---------- conv + extraction for one (slab, channel) ----------
        def emit_conv_cc(pmap, s, cc, mt):
            _, nin, nout = SLABS[s]
            ci, cl = cc // 4, cc % 4
            va = pmap[(0, ci)][0:nin, cl, :]
            vb = pmap[(1, ci)][0:nin, cl, :]
            for wh in (0, 1):
                b0 = wh * WC
                qg = psum.tile([128, 2048], F32, tag="qg", bufs=2)
                # weight-grouped: one LDWEIGHTS serves both tensors
                for w_name, off, st, sp in (
                        ("w_d", 1, True, False), ("w_d", 3, False, False),
                        ("w_d2", 2, False, True)):
                    for k, v in ((2, va), (3, vb)):
                        MM(qg[:, k * WC: (k + 1) * WC],
                           lhsT=w_sb[w_name][0:nin, :],
                           rhs=v[:, b0 + off: b0 + off + WC],
                           start=st, stop=sp)
                for w_name, off, st, sp in (
                        ("w_sp", 3, True, False), ("w_sn", 1, False, True)):
                    for k, v in ((0, va), (1, vb)):
                        MM(qg[:, k * WC: (k + 1) * WC],
                           lhsT=w_sb[w_name][0:nin, :],
                           rhs=v[:, b0 + off: b0 + off + WC],
                           start=st, stop=sp)
                g2 = sb.tile([128, 1024], BF16, tag="g2", bufs=3)
                nc.scalar.activation(out=g2[0:nout, :], in_=qg[0:nout, 0:1024],
                                     func=SQUARE)
                q = sb.tile([128, 1024], BF16, tag="q", bufs=3)
                col = next(qcol)
                nc.vector._custom_dve(
                    SQADD, out=q[0:nout, :], in0=qg[0:nout, 1024:2048],
                    in1=g2[0:nout, :], s0=0.0,
                    accum_out=acc_sb[0:nout, col: col + 1])
                nc.gpsimd.tensor_mul(out=mt[0:nout, cc, b0: b0 + WC],
                                     in0=q[0:nout, 0:WC],
                                     in1=q[0:nout, WC: 2 * WC])

        # ---------- prologue: slab0 build + remainder staging ----------
        cur_p, nxt_p = {}, {}
        for ti in (0, 1):
            for ci in range(5):
                emit_dma_exp(cur_p, 0, ti, ci)

        # remainder inputs: packed (c, r) partitions, one DMA per tensor
        rps = sb.tile([128, 2, W + 4], BF16, tag="rps", bufs=1)
        nc.vector.memset(rps[0:npk, :, 1: W + 3: W + 1], 0.0)
        rstg = sb.tile([128, 2, W], BF16, tag="rstg", bufs=1)
        for ti in (0, 1):
            nc.sync.dma_start(
                out=rstg[0:npk, ti, :],
                in_=x_dram[ti][REM[0]: REM[0] + blk, :, :]
                .rearrange("h c w -> c h w"))
        nc.scalar.activation(out=rps[0:npk, :, 2: 2 + W],
                             in_=rstg[0:npk, :, :], func=EXP)

        # z + reciprocal + first-chunk normalize for slab0; remaining chunks'
        # normalizes are injected into the cc loop just before they're needed
        # so VectorE never lumps them ahead of the SQADD stream.
        r16s = {}
        for ti in (0, 1):
            zt = emit_z(cur_p, 0, ti)
            r16s[(0, ti)] = emit_recip(0, ti, zt)
            emit_norm(cur_p, 0, ti, r16s[(0, ti)], (0,))

        # ---------- main slabs with pipelined next-slab build ----------
        for s in (0, 1):
            mt = sb.tile([128, C, W], BF16, tag="m", bufs=1)
            pops = {}

            def pin(cc, f):
                pops.setdefault(cc, []).append(f)

            def nrm(pmap, ps, ti, ci):
                return lambda: emit_norm(pmap, ps, ti, r16s[(ps, ti)], (ci,))

            # late normalizes of the current slab (chunk ci needed at cc=4*ci)
            cp = dict(cur_p)
            for ci in (1, 2, 3, 4):
                for ti in (0, 1):
                    pin(4 * ci - 4 + ti, nrm(cp, s, ti, ci))
            if s == 0:
                np_ = nxt_p

                def de(ti, ci):
                    return lambda: emit_dma_exp(np_, 1, ti, ci)

                def zr(ti):
                    def f():
                        zt = emit_z(np_, 1, ti)
                        r16s[(1, ti)] = emit_recip(1, ti, zt)
                        emit_norm(np_, 1, ti, r16s[(1, ti)], (0,))
                    return f

                for cc, f in ((2, de(0, 0)), (3, de(1, 0)), (4, de(0, 1)),
                              (5, de(1, 1)), (8, de(0, 2)), (9, de(1, 2)),
                              (12, de(0, 3)), (13, de(1, 3)), (16, de(0, 4)),
                              (17, de(1, 4)), (17, zr(0)), (18, zr(1))):
                    pin(cc, f)
            for cc in range(C):
                emit_conv_cc(cur_p, s, cc, mt)
                for f in pops.get(cc, ()):
                    f()
            sc = next(scol)
            _, _, nout = SLABS[s]
            nc.scalar.activation(out=mt[0:nout, :, :], in_=mt[0:nout, :, :],
                                 func=SQRT,
                                 accum_out=acc_sb[0:nout, sc: sc + 1])
            cur_p, nxt_p = nxt_p, {}

        # ---------- packed remainder compute ----------
        for ti in (0, 1):
            zq = psum.tile([128, 2048], F32, tag="qg", bufs=2)
            for wh in (0, 1):
                MM(zq[0:blk, wh * WC: (wh + 1) * WC],
                   lhsT=w_sb["w_sel"][0:npk, 0:blk],
                   rhs=rps[0:npk, ti, 2 + wh * WC: 2 + (wh + 1) * WC],
                   start=True, stop=True)
            r32 = sb.tile([128, W], F32, tag="r32", bufs=2)
            nc.vector.reciprocal_approx_fast(out=r32[0:blk, :],
                                             in_=zq[0:blk, 0:1024])
            r16 = sb.tile([128, W], BF16, tag="r16", bufs=2)
            nc.vector.tensor_copy(out=r16[0:blk, :], in_=r32[0:blk, :])
            rrep = psum.tile([128, 2048], F32, tag="qg", bufs=2)
            for wh in (0, 1):
                MM(rrep[0:npk, wh * WC: (wh + 1) * WC],
                   lhsT=w_sb["w_rep"][0:blk, 0:npk],
                   rhs=r16[0:blk, wh * WC: (wh + 1) * WC],
                   start=True, stop=True)
            nc.vector.tensor_mul(out=rps[0:npk, ti, 2: 2 + W],
                                 in0=rps[0:npk, ti, 2: 2 + W],
                                 in1=rrep[0:npk, 0:1024])
        mt = sb.tile([128, C, W], BF16, tag="m", bufs=1)
        q_tiles = []
        for ti in (0, 1):
            q = sb.tile([128, 1024], BF16, tag="q", bufs=3)
            for wh in (0, 1):
                b0 = wh * WC
                qg = psum.tile([128, 2048], F32, tag="qg", bufs=2)
                for w_name, off, st, sp, k in (
                        ("w_rd", 1, True, False, 2), ("w_rd", 3, False, False, 2),
                        ("w_rd2", 2, False, True, 2),
                        ("w_rsp", 3, True, False, 0), ("w_rsn", 1, False, True, 0)):
                    MM(qg[:, k * WC: (k + 1) * WC],
                       lhsT=w_sb[w_name][0:npk, :],
                       rhs=rps[0:npk, ti, b0 + off: b0 + off + WC],
                       start=st, stop=sp)
                g2 = sb.tile([128, 1024], BF16, tag="g2", bufs=3)
                nc.scalar.activation(out=g2[0:npk, 0:WC], in_=qg[0:npk, 0:WC],
                                     func=SQUARE)
                col = next(qcol)
                nc.vector._custom_dve(
                    SQADD, out=q[0:npk, b0: b0 + WC],
                    in0=qg[0:npk, 2 * WC: 3 * WC], in1=g2[0:npk, 0:WC], s0=0.0,
                    accum_out=acc_sb[0:npk, col: col + 1])
            q_tiles.append(q)
        nc.gpsimd.tensor_mul(out=mt[0:npk, 0, :], in0=q_tiles[0][0:npk, :],
                             in1=q_tiles[1][0:npk, :])
        sc = next(scol)
        nc.scalar.activation(out=mt[0:npk, 0, :], in_=mt[0:npk, 0, :],
                             func=SQRT, accum_out=acc_sb[0:npk, sc: sc + 1])

        nc.sync.dma_start(out=acc_out[:, :], in_=acc_sb[:, :])
    if not nc.is_finalized():
        nc.finalize()
    return nc


def shard_inputs(student_logits, teacher_logits, c_dim=C, h_dim=H, w_dim=W,
                 ncores=NCORES):
    """Full (B,C,H,W) fp32 -> per-core (rows+2, C, W) bf16 halo shards."""
    b_dim = student_logits.shape[0]
    rows = (b_dim * h_dim) // ncores
    wts = _band_weights()
    in_maps = []
    for k in range(ncores):
        g0 = k * rows
        bi, h0 = g0 // h_dim, g0 % h_dim
        m = {}
        for name, x in (("xs", student_logits), ("xt", teacher_logits)):
            img = np.asarray(x[bi], np.float32)            # (C, H, W)
            sh = np.zeros((rows + 2, c_dim, w_dim), NP_BF16)
            lo, hi = h0 - 1, h0 + rows + 1
            slo, shi = max(lo, 0), min(hi, h_dim)
            sh[slo - lo: slo - lo + (shi - slo)] = \
                img[:, slo:shi, :].transpose(1, 0, 2).astype(NP_BF16)
            m[name] = sh
        for name, wv in wts.items():
            m[name] = wv
        in_maps.append(m)
    return in_maps


_NC_CACHE = {}


def _get_nc():
    if "full" not in _NC_CACHE:
        _NC_CACHE["full"] = build_nc()
    return _NC_CACHE["full"]


def run_on_cores(in_maps, trace=False, **kw):
    nc = _get_nc()
    return bass_utils.run_bass_kernel_spmd(
        nc, in_maps, core_ids=list(range(len(in_maps))), trace=trace, **kw)


def finish(results, n_total=None):
    if n_total is None:
        n_total = B * C * H * W
    tq = 0.0
    tcross = 0.0
    for r in results:
        a = np.asarray(r["acc"], np.float64)
        tq += a[:, :NQ].sum()
        tcross += a[:, NQ:].sum()
    return np.float32((tq - 2.0 * tcross) / n_total)


def kernel(student_logits, teacher_logits):
    in_maps = shard_inputs(np.asarray(student_logits), np.asarray(teacher_logits))
    res = run_on_cores(in_maps)
    return finish(res.results)
